# revision 19
# baseline (speedup 1.0000x reference)
"""Multi-head GNN attention message-passing kernel for 8 TRN2 NeuronCores.

Node-major edge layout (v2):
  - Host: nodes sorted by (deg_lo, deg_hi) and packed 128-per-window so every
    window has near-uniform in-degree; windows dealt in blocks of 8 to cores
    so all cores share one per-slot (s_lo, s_hi) size profile (SPMD program).
  - Edge slots form a (c, n) grid per window: slot c*128+n holds the c-th
    edge of the window's n-th dst node, so dma_gather (idx position -> slot)
    lands each edge's K|V row in its dst node's partition.  Q then needs no
    per-edge gather/one-hot: it broadcasts along c from a resident Q tile.
  - Scores: kq = K (.) Q (2x DVE), log-tree reduce over d, pad-kill+clamp via
    a +20/-30000 mask (min), exp on ACT straight into the message tile.
  - Messages: mS = V' (.) score with V columns stored (d,h)-permuted so the
    head-broadcast is packed in the last dim (2x DVE).
  - Segment-sum: per-slot matmul with identity lhsT accumulating in PSUM
    (pads contribute exactly 0 because exp(-7500) == 0).
  - K-bias dropped (cancels in wV/z); V-bias added in the epilogue; Q-bias
    kept.  KV table rows are partition-major-permuted so phase-1 writes use
    2KB descriptors; gather indices absorb the permutation on host.
"""

import math
from dataclasses import dataclass

import numpy as np

P = 128
H = 8
D = 16
HD = H * D  # 128
IN_DIM = 128
LO_CAP = 32768
CB = 48  # max subtiles per chunk (bounds SBUF tiles; multiple of 3 for segsum)


def _qcol():
    # dev col q=(d*8+h) holds original col (h*16+d)
    return np.array([(q % 8) * 16 + q // 8 for q in range(128)], dtype=np.int64)


def _ocol():
    # original col o=(h*16+d) lives at dev col (d*8+h)
    return np.array([(o % 16) * 8 + o // 16 for o in range(128)], dtype=np.int64)


@dataclass(frozen=True)
class Cfg:
    n: int
    ncores: int
    nw: int                  # window slots per core
    profile: tuple           # ((s_lo, s_hi) per slot), shared by all cores
    gchunk: int = 7          # subtiles per dma_gather (896 idx; 56 desc/engine)
    nq: int = 4

    @property
    def nloc(self) -> int:
        return self.nw * P

    @property
    def np_(self) -> int:
        return self.nloc * self.ncores

    @property
    def nwg(self) -> int:
        return self.np_ // P

    @property
    def tot_lo(self) -> int:
        return sum(s for s, _ in self.profile)

    @property
    def tot_hi(self) -> int:
        return sum(s for _, s in self.profile)

    @property
    def tot_c(self) -> int:
        return self.tot_lo + self.tot_hi


def _wrap_idx(idx: np.ndarray) -> np.ndarray:
    """[num] -> [128, num//16] int16 in the dma_gather wrapped+replicated layout."""
    w = idx.astype(np.int16).reshape(-1, 16).T
    return np.tile(w, (8, 1))


def _bf16(a):
    import ml_dtypes

    return np.asarray(a, dtype=np.float32).astype(ml_dtypes.bfloat16)


def preprocess(h, Wq, bq, Wk, bk, Wv, bv, src, dst, ncores=8):
    n = h.shape[0]
    nloc = int(math.ceil(n / (ncores * P))) * P
    np_ = nloc * ncores
    nw = nloc // P
    nwg = np_ // P

    src = np.asarray(src).astype(np.int64)
    dst = np.asarray(dst).astype(np.int64)

    # row-major table: row = node id (enables lo-then-hi phase-1 fencing)
    prow = src.copy()
    is_hi = prow >= LO_CAP

    deg_lo = np.bincount(dst[~is_hi], minlength=np_)
    deg_hi = np.bincount(dst[is_hi], minlength=np_)

    # nodes sorted by (deg_lo, deg_hi); consecutive 128 -> one window
    order = np.lexsort((deg_hi, deg_lo))
    win_nodes = order.reshape(nwg, P)
    w_slo = deg_lo[win_nodes].max(axis=1)
    w_shi = deg_hi[win_nodes].max(axis=1)

    # group windows into blocks of ncores minimizing sum(max slo + max shi):
    # greedy dominated-fill (seed with biggest, fill with dominated windows)
    rem = set(range(nwg))
    blocks = []
    tot = w_slo + w_shi
    while rem:
        rl = np.array(sorted(rem))
        seed = rl[np.argmax(tot[rl])]
        blk = [seed]
        L, Hh = w_slo[seed], w_shi[seed]
        rem.discard(seed)
        for _ in range(ncores - 1):
            rl = np.array(sorted(rem))
            dom = rl[(w_slo[rl] <= L) & (w_shi[rl] <= Hh)]
            if len(dom):
                pick = dom[np.argmax(tot[dom])]
            else:
                inc = (np.maximum(w_slo[rl] - L, 0)
                       + np.maximum(w_shi[rl] - Hh, 0))
                pick = rl[np.argmin(inc)]
                L = max(L, w_slo[pick])
                Hh = max(Hh, w_shi[pick])
            blk.append(pick)
            rem.discard(pick)
        blocks.append(blk)
    worder = np.array(blocks).reshape(-1)  # worder[k*ncores+c] = window id
    slot_of_win = np.empty(nwg, np.int64)
    core_of_win = np.empty(nwg, np.int64)
    for k in range(nw):
        blk = worder[k * ncores:(k + 1) * ncores]
        slot_of_win[blk] = k
        core_of_win[blk] = np.arange(ncores)
    def _round3(sl, sh):
        # total subtiles per window must be a multiple of 3 (full segsum rounds)
        rem = (sl + sh) % 3
        if rem and sl + sh > 0:
            sh += 3 - rem
        return sl, sh

    profile = tuple(
        _round3(int(w_slo[worder[k * ncores:(k + 1) * ncores]].max()),
                int(w_shi[worder[k * ncores:(k + 1) * ncores]].max()))
        for k in range(nw)
    )
    cfg = Cfg(n=n, ncores=ncores, nw=nw, profile=profile)

    pads = sum((sl - deg_lo[win_nodes[w]]).sum() + (sh - deg_hi[win_nodes[w]]).sum()
               for w, (sl, sh) in ((w, profile[slot_of_win[w]]) for w in range(nwg)))
    e_real = len(src)
    print(f"[prep] slots/core={sum(a+b for a,b in profile)} pad_frac="
          f"{pads/(e_real+pads):.3f} cmax={max(a+b for a,b in profile)}")

    # node -> (core, slot, partition)
    node_core = np.empty(np_, np.int64)
    node_slot = np.empty(np_, np.int64)
    node_part = np.empty(np_, np.int64)
    node_core[win_nodes] = core_of_win[:, None]
    node_slot[win_nodes] = slot_of_win[:, None]
    node_part[win_nodes] = np.arange(P)[None, :]

    # per-edge slot position: rank within (dst, half)
    # sort edges by (dst, half) then rank within group
    okey = np.lexsort((is_hi, dst))
    d_s, h_s, p_s = dst[okey], is_hi[okey], prow[okey]
    grp = d_s * 2 + h_s
    new_grp = np.empty(len(grp), bool)
    new_grp[0] = True
    new_grp[1:] = grp[1:] != grp[:-1]
    gstart = np.where(new_grp)[0]
    rank = np.arange(len(grp)) - np.repeat(gstart, np.diff(np.append(gstart, len(grp))))

    core_e = node_core[d_s]
    slot_e = node_slot[d_s]
    part_e = node_part[d_s]

    lo_off = np.zeros(nw + 1, np.int64)
    hi_off = np.zeros(nw + 1, np.int64)
    for k in range(nw):
        lo_off[k + 1] = lo_off[k] + profile[k][0]
        hi_off[k + 1] = hi_off[k] + profile[k][1]

    tot_lo, tot_hi = cfg.tot_lo, cfg.tot_hi
    lo_idx = np.zeros((ncores, tot_lo * P), np.int64)
    hi_idx = np.zeros((ncores, max(tot_hi, 1) * P), np.int64)
    pos_lo = lo_off[slot_e] * P + rank * P + part_e
    pos_hi = hi_off[slot_e] * P + rank * P + part_e
    m_lo = ~h_s
    # scatter (vectorized per core)
    for c in range(ncores):
        mc = core_e == c
        ml = mc & m_lo
        mh = mc & h_s
        lo_idx[c][pos_lo[ml]] = p_s[ml]
        hi_idx[c][pos_hi[mh]] = p_s[mh] - LO_CAP

    # masks: +20 (clamp) for real slots, -30000 for pads; layout [P, tot_c]
    mask = np.full((ncores, P, cfg.tot_c), -30000.0, np.float32)
    for c in range(ncores):
        for k in range(nw):
            sl, sh = profile[k]
            co = lo_off[k] + hi_off[k]
            w = worder[k * ncores + c]
            dl = deg_lo[win_nodes[w]]
            dh = deg_hi[win_nodes[w]]
            if sl:
                mask[c, :, co:co + sl] = np.where(
                    np.arange(sl)[None, :] < dl[:, None], 20.0, -30000.0)
            if sh:
                mask[c, :, co + sl:co + sl + sh] = np.where(
                    np.arange(sh)[None, :] < dh[:, None], 20.0, -30000.0)

    f32 = np.float32
    qc = _qcol()
    hT = np.zeros((IN_DIM, np_), dtype=f32)
    hT[:, :n] = np.asarray(h, dtype=f32).T
    hTb = _bf16(hT)
    Wv_p = np.asarray(Wv, f32)[:, qc]
    bv_p = np.asarray(bv, f32)[qc]

    shared = {
        "hT": hTb,
        "Wkv": _bf16(np.hstack([np.asarray(Wk, f32), Wv_p])),
        "Wq_": _bf16(np.asarray(Wq, f32)),
        "bq_rep": _bf16(np.tile(np.asarray(bq, f32), (P, 1))),
        "bv_rep": _bf16(np.tile(bv_p, (P, 1))),
        "ident": _bf16(np.eye(P, dtype=f32)),
    }

    per_core = []
    node_order_all = []
    for c in range(ncores):
        sigma = win_nodes[worder[np.arange(nw) * ncores + c]].reshape(-1)  # [nloc]
        node_order_all.append(sigma)
        hTloc = np.zeros((IN_DIM, nloc), f32)
        real = sigma < n
        hTloc[:, real] = np.asarray(h, f32).T[:, sigma[real]]
        per_core.append({
            "kvloidx": _wrap_idx(lo_idx[c]),
            "kvhiidx": _wrap_idx(hi_idx[c]),
            "mask": _bf16(mask[c]),
            "hTloc": _bf16(hTloc),
        })
    return cfg, shared, per_core, np.concatenate(node_order_all)


def build_program(cfg: Cfg):
    import concourse.bacc as bacc
    import concourse.mybir as mybir
    import concourse.tile as tile

    F32 = mybir.dt.float32
    BF16 = mybir.dt.bfloat16
    I16 = mybir.dt.int16
    AO = mybir.AluOpType
    AF = mybir.ActivationFunctionType

    nc = bacc.Bacc(
        "TRN2",
        target_bir_lowering=False,
        debug=False,
        num_devices=cfg.ncores,
        num_swdge_queues=cfg.nq,
    )

    np_, nloc, nw, nwg = cfg.np_, cfg.nloc, cfg.nw, cfg.nwg

    hT_d = nc.dram_tensor("hT", [IN_DIM, np_], BF16, kind="ExternalInput")
    hTloc_d = nc.dram_tensor("hTloc", [IN_DIM, nloc], BF16, kind="ExternalInput")
    Wkv_d = nc.dram_tensor("Wkv", [IN_DIM, 2 * HD], BF16, kind="ExternalInput")
    Wq_d = nc.dram_tensor("Wq_", [IN_DIM, HD], BF16, kind="ExternalInput")
    bqr_d = nc.dram_tensor("bq_rep", [P, HD], BF16, kind="ExternalInput")
    bvr_d = nc.dram_tensor("bv_rep", [P, HD], BF16, kind="ExternalInput")
    ident_d = nc.dram_tensor("ident", [P, P], BF16, kind="ExternalInput")
    kvlo_i_d = nc.dram_tensor("kvloidx", [P, cfg.tot_lo * 8], I16, kind="ExternalInput")
    kvhi_i_d = nc.dram_tensor(
        "kvhiidx", [P, max(cfg.tot_hi, 1) * 8], I16, kind="ExternalInput")
    mask_d = nc.dram_tensor("mask", [P, cfg.tot_c], BF16, kind="ExternalInput")
    out_d = nc.dram_tensor("out", [nloc, HD], F32, kind="ExternalOutput")

    # KV table, row r = (node%128)*nwg + node//128
    KV_d = nc.dram_tensor("KV", [np_, 2 * HD], BF16, kind="Internal")

    _swdge_ctr = [0]

    def chunked_gather(view_d, idx_t, kv3, sub_off, idx_off, nsub, fence):
        off = 0
        while off < nsub:
            gc = min(cfg.gchunk, nsub - off)
            ga = nc.gpsimd.dma_gather(
                out_ap=kv3[:, sub_off + off: sub_off + off + gc, :],
                in_ap=view_d,
                idxs_ap=idx_t[:, (idx_off + off) * 8: (idx_off + off + gc) * 8],
                num_idxs=gc * P,
                num_idxs_reg=gc * P,
                elem_size=2 * HD,
                single_packet=True,
                queue_num=_swdge_ctr[0] % cfg.nq,
            )
            tile.add_dep_helper(ga.ins, fence.ins, reason="gather>kv")
            _swdge_ctr[0] += 1
            off += gc

    kv_writes_lo = []
    kv_writes_hi = []

    with tile.TileContext(nc) as tc:
        with (
            tc.tile_pool(name="consts", bufs=1) as p_c,
            tc.tile_pool(name="p1", bufs=4) as p_1,
            tc.tile_pool(name="gath", bufs=3) as p_g,
            tc.tile_pool(name="work", bufs=2) as p_wk,
            tc.tile_pool(name="epi", bufs=2) as p_epi,
        ):
            wkv_t = p_c.tile([P, 2 * HD], BF16)
            nc.sync.dma_start(out=wkv_t[:], in_=Wkv_d[:, :])
            wq_t = p_c.tile([P, HD], BF16)
            nc.sync.dma_start(out=wq_t[:], in_=Wq_d[:, :])
            bqr_t = p_c.tile([P, HD], BF16)
            nc.sync.dma_start(out=bqr_t[:], in_=bqr_d[:, :])
            bvr_t = p_c.tile([P, HD], BF16)
            nc.sync.dma_start(out=bvr_t[:], in_=bvr_d[:, :])
            ident_t = p_c.tile([P, P], BF16)
            nc.sync.dma_start(out=ident_t[:], in_=ident_d[:, :])
            q_all = p_c.tile([P, nw * HD], BF16)
            zero_out = p_c.tile([P, HD], F32)
            nc.vector.memset(zero_out[:], 0.0)

            # ---- phase 1: K|V' for all nodes -> partition-major HBM table ----
            p_1ps_cm = tc.tile_pool(name="p1ps", bufs=2, space="PSUM")
            p_1ps = p_1ps_cm.__enter__()
            for g4 in range(0, nwg, 4):
                gn = min(4, nwg - g4)
                ht4 = p_1.tile([P, 4 * P], BF16, tag="ht")
                nc.sync.dma_start(
                    out=ht4[:, : gn * P], in_=hT_d[:, g4 * P:(g4 + gn) * P])
                ps4p = p_1ps.tile([P, 4 * 2 * HD], F32, tag="p1ps")
                for j in range(gn):
                    nc.tensor.matmul(
                        out=ps4p[:, j * 2 * HD:(j + 1) * 2 * HD],
                        lhsT=ht4[:, j * P:(j + 1) * P], rhs=wkv_t[:],
                        start=True, stop=True)
                kv_sb4 = p_1.tile([P, 4 * 2 * HD], BF16, tag="kvsb")
                nc.scalar.activation(
                    out=kv_sb4[:, : gn * 2 * HD], in_=ps4p[:, : gn * 2 * HD],
                    func=AF.Copy)
                wr = nc.sync.dma_start(
                    out=KV_d[g4 * P:(g4 + gn) * P, :].rearrange(
                        "(j p) e -> p j e", p=P),
                    in_=kv_sb4[:].rearrange("p (j e) -> p j e", e=2 * HD)[:, :gn, :])
                (kv_writes_lo if (g4 + gn) * P <= LO_CAP
                 else kv_writes_hi).append(wr)

            # ---- phase 1b: Q + bq for the local (permuted) node order ----
            for w4 in range(0, nw, 4):
                wn = min(4, nw - w4)
                ht4 = p_1.tile([P, 4 * P], BF16, tag="ht")
                nc.sync.dma_start(
                    out=ht4[:, : wn * P], in_=hTloc_d[:, w4 * P:(w4 + wn) * P])
                psq4 = p_1ps.tile([P, 4 * 2 * HD], F32, tag="p1ps")
                for j in range(wn):
                    nc.tensor.matmul(
                        out=psq4[:, j * 2 * HD: j * 2 * HD + HD],
                        lhsT=ht4[:, j * P:(j + 1) * P], rhs=wq_t[:],
                        start=True, stop=True)
                for j in range(wn):
                    w = w4 + j
                    nc.vector.tensor_tensor(
                        out=q_all[:, w * HD:(w + 1) * HD],
                        in0=psq4[:, j * 2 * HD: j * 2 * HD + HD],
                        in1=bqr_t[:], op=AO.add)

            p_1ps_cm.__exit__(None, None, None)
            kv_fence_lo = nc.sync.nop()
            for wr in kv_writes_lo:
                tile.add_dep_helper(kv_fence_lo.ins, wr.ins, reason="kv lo fence")
            kv_fence_hi = nc.sync.nop()
            for wr in kv_writes_hi:
                tile.add_dep_helper(kv_fence_hi.ins, wr.ins, reason="kv hi fence")

            KVlo_v = KV_d[0:LO_CAP, :]
            KVhi_v = KV_d[LO_CAP:np_, :]

            p_2ps_cm = tc.tile_pool(name="p2ps", bufs=2, space="PSUM")
            p_2ps = p_2ps_cm.__enter__()
            NG = 3  # segsum groups; 3*(HD+H) f32 = 1632B fits one psum bank

            def emit_main(k, lo_off, hi_off, c_off):
                """Gathers + score/message compute + segsum for window k.
                Returns state for the deferred epilogue."""
                slo, shi = cfg.profile[k]
                c = slo + shi
                qw = q_all[:, k * HD:(k + 1) * HD]
                il_t = p_g.tile([P, max(slo, 1) * 8], I16, tag="il", bufs=2)
                if slo:
                    nc.sync.dma_start(
                        out=il_t[:, : slo * 8],
                        in_=kvlo_i_d[:, lo_off * 8:(lo_off + slo) * 8])
                ih_t = p_g.tile([P, max(shi, 1) * 8], I16, tag="ih", bufs=2)
                if shi:
                    nc.sync.dma_start(
                        out=ih_t[:, : shi * 8],
                        in_=kvhi_i_d[:, hi_off * 8:(hi_off + shi) * 8])
                mk_t = p_g.tile([P, c], BF16, tag="mk", bufs=2)
                nc.sync.dma_start(out=mk_t[:], in_=mask_d[:, c_off:c_off + c])

                kv_t = p_g.tile([P, c * 2 * HD], BF16, tag="kv", bufs=2)
                kv3 = kv_t[:].rearrange("p (s e) -> p s e", e=2 * HD)
                if slo:
                    chunked_gather(KVlo_v, il_t, kv3, 0, 0, slo, kv_fence_lo)
                if shi:
                    chunked_gather(KVhi_v, ih_t, kv3, slo, 0, shi, kv_fence_hi)

                ps4 = p_2ps.tile([P, HD + H], F32, tag="ps4", bufs=3)
                nchunk = (c + CB - 1) // CB
                for ci in range(nchunk):
                    c0 = ci * CB
                    cb = min(CB, c - c0)
                    kvc = kv3[:, c0:c0 + cb, :]
                    kq = p_wk.tile([P, CB * HD], BF16, tag="kq")
                    kq3 = kq[:, : cb * HD].rearrange("p (s e) -> p s e", e=HD)
                    nc.vector.tensor_tensor(
                        out=kq3, in0=kvc[:, :, 0:HD],
                        in1=qw.unsqueeze(1).to_broadcast([P, cb, HD]),
                        op=AO.mult)
                    kqv = kq[:, : cb * HD].rearrange("p (x d) -> p x d", d=16)
                    t1 = p_wk.tile([P, CB * 64], BF16, tag="t1")
                    t1v = t1[:, : cb * 64].rearrange("p (x d) -> p x d", d=8)
                    nc.vector.tensor_tensor(
                        out=t1v, in0=kqv[:, :, 0:8], in1=kqv[:, :, 8:16], op=AO.add)
                    t2 = p_wk.tile([P, CB * 32], BF16, tag="t2")
                    t2v = t2[:, : cb * 32].rearrange("p (x d) -> p x d", d=4)
                    nc.vector.tensor_tensor(
                        out=t2v, in0=t1v[:, :, 0:4], in1=t1v[:, :, 4:8], op=AO.add)
                    t3 = p_wk.tile([P, CB * 16], BF16, tag="t3")
                    t3v = t3[:, : cb * 16].rearrange("p (x d) -> p x d", d=2)
                    nc.vector.tensor_tensor(
                        out=t3v, in0=t2v[:, :, 0:2], in1=t2v[:, :, 2:4], op=AO.add)
                    sraw = p_wk.tile([P, CB * H], BF16, tag="sraw")
                    nc.vector.tensor_tensor(
                        out=sraw[:, : cb * H], in0=t3v[:, :, 0], in1=t3v[:, :, 1],
                        op=AO.add)
                    smsk = p_wk.tile([P, CB * H], BF16, tag="smsk")
                    nc.vector.tensor_tensor(
                        out=smsk[:, : cb * H].rearrange("p (s h) -> p s h", h=H),
                        in0=sraw[:, : cb * H].rearrange("p (s h) -> p s h", h=H),
                        in1=mk_t[:, c0:c0 + cb].unsqueeze(2).to_broadcast(
                            [P, cb, H]),
                        op=AO.min)
                    mS = p_wk.tile([P, CB * (HD + H)], BF16, tag="mS")
                    mS3 = mS[:, : cb * (HD + H)].rearrange(
                        "p (s f) -> p s f", f=HD + H)
                    nc.scalar.activation(
                        out=mS3[:, :, HD:HD + H],
                        in_=smsk[:, : cb * H].rearrange("p (s h) -> p s h", h=H),
                        func=AF.Exp, scale=0.25)
                    nc.vector.tensor_tensor(
                        out=mS3[:, :, 0:HD].rearrange("p s (d h) -> p s d h", h=H),
                        in0=kvc[:, :, HD:2 * HD].rearrange(
                            "p s (d h) -> p s d h", h=H),
                        in1=mS3[:, :, HD:HD + H].unsqueeze(2).to_broadcast(
                            [P, cb, D, H]),
                        op=AO.mult)
                    # segsum: single psum accumulation chain
                    for j in range(cb):
                        g = c0 + j
                        nc.tensor.matmul(
                            out=ps4[:],
                            lhsT=ident_t[:],
                            rhs=mS3[:, j, :],
                            start=(g == 0), stop=(g == c - 1),
                        )
                return ps4, c, k

            def emit_epi(state):
                ps4, c, k = state
                zr = p_epi.tile([P, H], F32, tag="zr")
                nc.vector.tensor_scalar_add(
                    out=zr[:], in0=ps4[:, HD:HD + H], scalar1=1e-6)
                nc.vector.reciprocal(out=zr[:], in_=zr[:])
                outsb = p_epi.tile([P, HD], F32, tag="outsb")
                nc.vector.tensor_tensor(
                    out=outsb[:].rearrange("p (d h) -> p d h", h=H),
                    in0=ps4[:, 0:HD].rearrange("p (d h) -> p d h", h=H),
                    in1=zr[:].unsqueeze(1).to_broadcast([P, D, H]),
                    op=AO.mult)
                nc.vector.tensor_tensor(
                    out=outsb[:], in0=outsb[:], in1=bvr_t[:], op=AO.add)
                nc.sync.dma_start(out=out_d[k * P:(k + 1) * P, :], in_=outsb[:])

            # software-pipelined: epilogue of window k-1 emitted after main(k)
            pending = None
            lo_off = hi_off = c_off = 0
            for k in range(nw):
                slo, shi = cfg.profile[k]
                c = slo + shi
                if c == 0:
                    nc.sync.dma_start(
                        out=out_d[k * P:(k + 1) * P, :], in_=zero_out[:])
                    continue
                state = emit_main(k, lo_off, hi_off, c_off)
                if pending is not None:
                    emit_epi(pending)
                pending = state
                lo_off += slo
                hi_off += shi
                c_off += c
            if pending is not None:
                emit_epi(pending)

            p_2ps_cm.__exit__(None, None, None)

    nc.compile()
    return nc


_CACHE: dict = {}


def _get_program(cfg: Cfg):
    if cfg not in _CACHE:
        _CACHE[cfg] = build_program(cfg)
    return _CACHE[cfg]


def run(h, Wq, bq, Wk, bk, Wv, bv, src, dst, trace=False, **run_kwargs):
    from concourse.bass_utils import run_bass_kernel_spmd

    h = np.asarray(h)
    cfg, shared, per_core, node_order = preprocess(
        h, np.asarray(Wq), np.asarray(bq), np.asarray(Wk), np.asarray(bk),
        np.asarray(Wv), np.asarray(bv), np.asarray(src), np.asarray(dst),
    )
    nc = _get_program(cfg)
    in_maps = [dict(shared, **pc) for pc in per_core]
    res = run_bass_kernel_spmd(
        nc, in_maps, core_ids=list(range(cfg.ncores)), trace=trace, **run_kwargs
    )
    outs = np.concatenate(
        [res.results[c]["out"] for c in range(cfg.ncores)], axis=0)
    full = np.zeros((cfg.np_, HD), np.float32)
    full[node_order] = outs.astype(np.float32)
    return full[: cfg.n][:, _ocol()], res


def kernel(h, Wq, bq, Wk, bk, Wv, bv, src, dst, **_):
    out, _res = run(h, Wq, bq, Wk, bk, Wv, bv, src, dst, trace=False)
    return out
